# revision 1
# baseline (speedup 1.0000x reference)
"""Trainium2 Bass kernel for the diag-conv problem.

Math (full problem, NET_SUM=512, K=512):
    P[i,r,c]  = X[i,r,c] * W[c,r]                (elementwise vs W^T)
    d1[i,r]   = sum_c P[i,r,c]                   (row sums)
    d2[i,c]   = sum_r P[i,r,c]                   (col sums)
    d         = d1 + d2
    out[i,r,c] = relu(0.1*(d[i,r] + d[i,c]))

Sharding: data-parallel over the batch axis i across 8 cores (64 each).
W^T (pre-scaled by 0.1) is replicated; all math on-device is fp32.

Per-core engine mapping (per batch element):
    DMA : batch elements stream in/out PAIR at a time via HWDGE (nc.sync)
    DVE : scalar_tensor_tensor -> P (SBUF) + d1 per-partition (fused pass)
    PE  : ones-matmuls reduce P over partitions -> d2 row in PSUM;
          4 transposes of the d1 accumulator -> d1 row in PSUM;
          ones^T (x) G -> B[p,f] = G[f] broadcast; 4 tiny k=1 matmuls ->
          G chunks as per-partition bias columns
    ACT : d1-row PSUM->SBUF copy; relu(B + bias_chunk) -> output tile
"""

import numpy as np

N_CORES = 8
NET_SUM = 512
K = 512
NB = NET_SUM // N_CORES  # 64 batches per core
NT = 4                   # 512 rows = 4 groups of 128 partitions
P_DIM = 128

# dtype of the stored P product / d2 column-sum matmul inputs
D2_DT = "float32"
# replay the whole batch loop this many times inside one NEFF (timing only)
REPEAT = 1
# batch elements moved per DMA (bigger -> fewer, larger transfers)
PAIR = 4
# row->partition mapping: False = partition p holds rows {p, 128+p, ...}
# (2KB contiguous runs per partition); True = partition p holds rows
# {4p..4p+3} (8KB contiguous runs per partition per batch element)
INTERLEAVE = False
# timing experiment: skip all compute, only DMA in/out
DMA_ONLY = False
# issue output DMAs from the ACT HWDGE ring (SP ring handles loads)
SPLIT_RINGS = False
# override xp/op pool depth (None = default 3)
BUFS = None

_CACHE = {}


def build(n_batch=NB, loop_n=1):
    from contextlib import nullcontext

    import concourse.mybir as mybir
    import concourse.tile as tile
    from concourse import bacc
    from concourse.masks import make_identity

    f32 = mybir.dt.float32
    d2_dt = getattr(mybir.dt, D2_DT)

    nc = bacc.Bacc("TRN2", target_bir_lowering=False, debug=False)

    x_dram = nc.dram_tensor("x4", [n_batch, NET_SUM, K], f32, kind="ExternalInput")
    wt_dram = nc.dram_tensor("wt", [P_DIM, NT, K], f32, kind="ExternalInput")
    out_dram = nc.dram_tensor(
        "out4", [n_batch, NET_SUM, K], f32, kind="ExternalOutput"
    )

    pair = PAIR
    assert n_batch % pair == 0
    big_bufs = (BUFS or 3) if pair <= 2 else 2

    if INTERLEAVE:
        # partition p <- rows 4p..4p+3 : contiguous 8KB per partition
        dram_pat = "b (p q) f -> p b q f"
        dram_kw = {"p": P_DIM}
        # row r = 4p + q ; row-position view of a [1, 512] row AP
        row_pat, row_kw = "o (p q) -> o q p", {"q": NT}
    else:
        dram_pat = "b (q p) f -> p b q f"
        dram_kw = {"q": NT}
        row_pat, row_kw = "o (q p) -> o q p", {"p": P_DIM}

    with tile.TileContext(nc) as tc:
        with (
            tc.tile_pool(name="const", bufs=1) as const_pool,
            tc.tile_pool(name="xp", bufs=big_bufs) as xp,
            tc.tile_pool(name="pp", bufs=(BUFS or 3)) as pp,
            tc.tile_pool(name="op", bufs=big_bufs) as op,
            tc.tile_pool(name="small", bufs=4) as small,
            tc.tile_pool(name="gps", bufs=2, space="PSUM") as gps,
            tc.tile_pool(name="dps", bufs=2, space="PSUM") as dps,
            tc.tile_pool(name="bps", bufs=2, space="PSUM") as bps,
            tc.tile_pool(name="cps", bufs=2, space="PSUM") as cps,
        ):
            wt = const_pool.tile([P_DIM, NT, K], f32)
            nc.sync.dma_start(wt[:], wt_dram[:])

            identity = const_pool.tile([P_DIM, P_DIM], f32)
            make_identity(nc, identity[:])

            ones_col = const_pool.tile([P_DIM, 1], d2_dt)
            nc.vector.memset(ones_col[:], 1.0)
            ones_row = const_pool.tile([1, P_DIM], f32)
            nc.vector.memset(ones_row[:], 1.0)
            one11 = const_pool.tile([1, 1], f32)
            nc.vector.memset(one11[:], 1.0)

            loop_ctx = tc.For_i(0, loop_n, 1) if loop_n > 1 else nullcontext()
            with loop_ctx:
                for ip in [
                    i for _ in range(REPEAT) for i in range(n_batch // pair)
                ]:
                    # one DMA moves `pair` batch elements (pair MB)
                    xpair = xp.tile([P_DIM, pair, NT, K], f32)
                    nc.sync.dma_start(
                        xpair[:],
                        x_dram[:][ip * pair : (ip + 1) * pair].rearrange(
                            dram_pat, **dram_kw
                        ),
                    )
                    out_eng = nc.scalar if SPLIT_RINGS else nc.sync
                    if DMA_ONLY:
                        out_eng.dma_start(
                            out_dram[:][ip * pair : (ip + 1) * pair].rearrange(
                                dram_pat, **dram_kw
                            ),
                            xpair[:],
                        )
                        continue
                    opair = op.tile([P_DIM, pair, NT, K], f32)
                    for j in range(pair):
                        x = xpair[:, j]
                        o = opair[:, j]
                        # P = x * wt ; d1 per-partition sums (fused DVE pass;
                        # tensor_tensor_reduce crashes the DVE on this HW)
                        p = pp.tile([P_DIM, NT, K], d2_dt)
                        d1 = small.tile([P_DIM, NT], f32, tag="d1")
                        for t in range(NT):
                            nc.vector.scalar_tensor_tensor(
                                out=p[:, t, :],
                                in0=x[:, t, :],
                                scalar=1.0,
                                in1=wt[:, t, :],
                                op0=mybir.AluOpType.mult,
                                op1=mybir.AluOpType.mult,
                                accum_out=d1[:, t : t + 1],
                            )

                        # d2 row [1,512] (column sums): clean 4-matmul
                        # PSUM accumulation group
                        psum_g = gps.tile([1, K], f32)
                        for t in range(NT):
                            nc.tensor.matmul(
                                psum_g[:, :],
                                ones_col[:],
                                p[:, t, :],
                                start=(t == 0),
                                stop=(t == NT - 1),
                            )
                        # d1 row [1,512]: 4 independent PE transposes of the
                        # d1 accumulator columns into row positions
                        psum_d1r = dps.tile([1, K], f32)
                        d1r_rows = psum_d1r[:].rearrange(row_pat, **row_kw)
                        for t in range(NT):
                            nc.tensor.matmul(
                                d1r_rows[:, t, :],
                                d1[:, t : t + 1],
                                identity[:],
                                is_transpose=True,
                                start=True,
                                stop=True,
                                skip_group_check=True,
                            )
                        # G = d1 + d2 (DVE reads only one PSUM operand, so
                        # the d1 row goes through SBUF via an ACT copy)
                        d1row = small.tile([1, K], f32, tag="d1row")
                        nc.scalar.copy(d1row[:], psum_d1r[:])
                        g = small.tile([1, K], f32, tag="g")
                        nc.vector.tensor_tensor(
                            out=g[:],
                            in0=psum_g[:],
                            in1=d1row[:],
                            op=mybir.AluOpType.add,
                        )

                        # B[p,f] = G[f] for all p (rank-1 broadcast matmul)
                        psum_b = bps.tile([P_DIM, K], f32)
                        nc.tensor.matmul(
                            psum_b[:],
                            ones_row[:],
                            g[:],
                            start=True,
                            stop=True,
                        )

                        # bias columns: gcol[p, t] = G[row(p, t)]
                        g_rows = g[:].rearrange(row_pat, **row_kw)
                        psum_gc = cps.tile([P_DIM, NT], f32)
                        for t in range(NT):
                            nc.tensor.matmul(
                                psum_gc[:, t : t + 1],
                                g_rows[:, t, :],
                                one11[:],
                                start=True,
                                stop=True,
                            )
                        gcol = small.tile([P_DIM, NT], f32, tag="gcol")
                        nc.scalar.copy(gcol[:], psum_gc[:])

                        # o[p, t, f] = relu(B[p,f] + gcol[p,t])
                        for t in range(NT):
                            nc.scalar.activation(
                                out=o[:, t, :],
                                in_=psum_b[:],
                                func=mybir.ActivationFunctionType.Relu,
                                bias=gcol[:, t : t + 1],
                                scale=1.0,
                            )

                    out_eng.dma_start(
                        out_dram[:][ip * pair : (ip + 1) * pair].rearrange(
                            dram_pat, **dram_kw
                        ),
                        opair[:],
                    )

    nc.compile()
    return nc


def _prep_host(input_feature, kernel):
    x = np.ascontiguousarray(np.asarray(input_feature, dtype=np.float32))
    w = np.asarray(kernel, dtype=np.float32)
    a = (0.1 * w.T).astype(np.float32)  # a[r, j] = 0.1 * w[j, r]
    if INTERLEAVE:
        wt = np.ascontiguousarray(a.reshape(P_DIM, NT, K))
    else:
        wt = np.ascontiguousarray(a.reshape(NT, P_DIM, K).transpose(1, 0, 2))
    x4 = x.reshape(N_CORES, NB, NET_SUM, K)
    return x4, wt


TRACE = False
LAST_RESULTS = None


def kernel(input_feature, kernel):
    global LAST_RESULTS
    from concourse.bass_utils import run_bass_kernel_spmd

    x4, wt = _prep_host(input_feature, kernel)

    if "nc" not in _CACHE:
        _CACHE["nc"] = build()
    nc = _CACHE["nc"]

    in_maps = [{"x4": np.ascontiguousarray(x4[c]), "wt": wt} for c in range(N_CORES)]
    res = run_bass_kernel_spmd(nc, in_maps, core_ids=list(range(N_CORES)), trace=TRACE)
    LAST_RESULTS = res
    out = np.concatenate([r["out4"] for r in res.results], axis=0)
    return out

